# revision 10
# baseline (speedup 1.0000x reference)
"""RWKV7 block kernel for 8 Trainium2 NeuronCores (data-parallel over batch).

Layouts per core (one batch row per core; T=2048, D=FFN=1024):
  T-major: [t-partition (16 chunks x 128), channel-free (1024)]
  C-major: [c-partition (8 chunks x 128), time-free (2048)]

Matmuls run in float32r (fp32 inputs rounded to ~9-bit mantissa, fp32 PSUM
accumulation, full PE rate); the recurrence is an exact fp32
tensor_tensor_scan along the time (free) dimension in C-major layout.

Pipeline:
  P0: LN1 stats+normalize (T-major) -> PE transpose -> xm^T = n_t + n_{t-1}
  P1: six projections (weight-stationary, C-major out): srg=sig(r)sig(g)->DRAM,
      u = (k+bk)(v+bv)+(a+ba)(b+bb) -> DRAM   (bk.. = ln1_b @ W folds)
  P2: scan s_t = w*s_{t-1} + u_t per channel -> states (SBUF)
  P3: transpose states -> GroupNorm(32-ch groups, stats on PSUM) -> transpose
      back -> gate m = (norm*gn_w[+gn_b])*srg  (C-major f32r, SBUF)
  P4: mixed = m @ Wout (T-major); res = x + mixed -> DRAM; LN2 -> transpose
      -> xx2^T (f32r; LN2 affine folded into Wfk/Wfr host-side)
  P5: kf = relu(xx2@Wfk'+bfk')^2 (SBUF); fr = sig(xx2@Wfr'+bfr') -> DRAM;
      h = kf@Wfv+bfv; [GRN if grn_gamma/beta nonzero]; g2 = fr*h;
      out = res + g2 @ Wffnout
"""
import sys
import os

for _p in ("/opt/trn_rl_repo", "/root/.axon_site/_ro/trn_rl_repo"):
    if os.path.isdir(_p) and _p not in sys.path:
        sys.path.append(_p)

import numpy as np
import concourse.bass as bass
from concourse import bacc
import concourse.tile as tile
from concourse import mybir
from concourse import bass_utils
from concourse.alu_op_type import AluOpType
from contextlib import ExitStack

AF = mybir.ActivationFunctionType
F32 = mybir.dt.float32
F32R = mybir.dt.float32r

B, T, D, FFN = 8, 2048, 1024, 1024
NT, NC = T // 128, D // 128
EPS_LN = 1e-5
EPS_GN = 1e-5

_CACHE = {}
_LAST_IN_MAPS = None


def _ts(i, n):
    return slice(i * n, (i + 1) * n)


def _build(use_grn: bool, use_gnb: bool):
    nc = bacc.Bacc("TRN2", target_bir_lowering=False, debug=False)

    x_in = nc.dram_tensor("x_in", [T, D], F32, kind="ExternalInput").ap()
    consts = nc.dram_tensor("consts", [128, 64], F32, kind="ExternalInput").ap()
    consts2 = nc.dram_tensor("consts2", [128, 64], F32, kind="ExternalInput").ap()
    ident_in = nc.dram_tensor("ident", [128, 128], F32, kind="ExternalInput").ap()
    w_names = ["wr", "wk", "wv", "wa", "wb", "wg", "wout", "wfk", "wfr", "wfv", "wffn"]
    w_dram = {n: nc.dram_tensor(n, [1024, 1024], F32, kind="ExternalInput").ap()
              for n in w_names}
    out_x = nc.dram_tensor("out_x", [T, D], F32, kind="ExternalOutput").ap()
    out_fs = nc.dram_tensor("out_fs", [128, NC], F32, kind="ExternalOutput").ap()

    with tile.TileContext(nc) as tc, ExitStack() as top:
        dram = top.enter_context(tc.tile_pool(name="dram", bufs=1, space="DRAM"))
        srg_d = dram.tile([D, T], F32)
        u1_d = dram.tile([D, T], F32)
        u_d = dram.tile([D, T], F32)
        fr_d = dram.tile([FFN, T], F32)
        res_d = dram.tile([T, D], F32)
        h_d = dram.tile([D, T], F32) if use_grn else None

        cpool = top.enter_context(tc.tile_pool(name="cpool", bufs=1))
        ct = cpool.tile([128, 64], F32)
        nc.sync.dma_start(ct[:], consts)
        c2 = cpool.tile([128, 64], F32)
        nc.sync.dma_start(c2[:], consts2)
        idt = cpool.tile([128, 128], F32)
        nc.sync.dma_start(idt[:], ident_in)
        PCOL = ct[:, 0:8]
        WSCAN = ct[:, 8:16]
        STATE0 = ct[:, 16:24]
        GNW = ct[:, 24:32]
        GNB = ct[:, 32:40]
        BK = ct[:, 40:48]
        BV = ct[:, 48:56]
        BETA = ct[:, 56:64]
        BA = c2[:, 0:8]
        BB = c2[:, 8:16]
        BR = c2[:, 16:24]
        BG = c2[:, 24:32]
        BFK = c2[:, 32:40]
        BFR = c2[:, 40:48]
        BFV = c2[:, 48:56]
        GAMMA = c2[:, 56:64]

        # slot A rotates: xmr -> normT -> xx2 -> g2 (64KB/partition)
        big1 = top.enter_context(tc.tile_pool(name="big1", bufs=1))

        # ---------------- P0 ----------------
        xmr = big1.tile([128, NC * T], F32R, tag="slotA", name="xmr")
        xmr_v = xmr[:].rearrange("p (c t) -> p c t", c=NC)

        def ln_rstd(pool, src_ap, scratch_ap, tag_pfx):
            """LN stats over 1024 free elems -> (rstd [128,1], -mean*rstd [128,1]).
            scratch_ap is clobbered with src^2 (pass a tile that gets
            overwritten later anyway)."""
            sm = pool.tile([128, 1], F32, tag=f"{tag_pfx}sm")
            nc.vector.tensor_reduce(sm[:], src_ap, mybir.AxisListType.X, AluOpType.add)
            ssq = pool.tile([128, 1], F32, tag=f"{tag_pfx}ssq")
            nc.scalar.activation(scratch_ap, src_ap, AF.Square, accum_out=ssq[:])
            mean = pool.tile([128, 1], F32, tag=f"{tag_pfx}mean")
            nc.vector.tensor_scalar(out=mean[:], in0=sm[:], scalar1=1.0 / D,
                                    scalar2=None, op0=AluOpType.mult)
            msq = pool.tile([128, 1], F32, tag=f"{tag_pfx}msq")
            nc.vector.tensor_tensor(out=msq[:], in0=mean[:], in1=mean[:],
                                    op=AluOpType.mult)
            v = pool.tile([128, 1], F32, tag=f"{tag_pfx}v")
            nc.vector.scalar_tensor_tensor(out=v[:], in0=ssq[:], scalar=1.0 / D,
                                           in1=msq[:], op0=AluOpType.mult,
                                           op1=AluOpType.subtract)
            nc.vector.tensor_scalar(out=v[:], in0=v[:], scalar1=EPS_LN,
                                    scalar2=None, op0=AluOpType.add)
            nc.vector.reciprocal(v[:], v[:])
            rstd = pool.tile([128, 1], F32, tag=f"{tag_pfx}rstd")
            nc.scalar.activation(rstd[:], v[:], AF.Sqrt)
            nmr = pool.tile([128, 1], F32, tag=f"{tag_pfx}nmr")
            nc.vector.scalar_tensor_tensor(out=nmr[:], in0=mean[:], scalar=-1.0,
                                           in1=rstd[:], op0=AluOpType.mult,
                                           op1=AluOpType.mult)
            return rstd, nmr

        with ExitStack() as ctx:
            p0 = ctx.enter_context(tc.tile_pool(name="p0", bufs=2))
            p0s = ctx.enter_context(tc.tile_pool(name="p0s", bufs=2))
            p0ps = ctx.enter_context(tc.tile_pool(name="p0ps", bufs=2, space="PSUM"))
            prevp = ctx.enter_context(tc.tile_pool(name="prevp", bufs=1))
            prevcol = prevp.tile([128, NC], F32)
            nc.vector.tensor_copy(prevcol[:], PCOL)
            pcv = prevcol[:].rearrange("p (c o) -> p c o", o=1)
            for i in range(NT):
                xt = p0.tile([128, D], F32, tag="xt")
                nc.sync.dma_start(xt[:], x_in[_ts(i, 128), :])
                nt_ = p0.tile([128, D], F32, tag="nt")
                rstd, nmr = ln_rstd(p0s, xt[:], nt_[:], "l1")
                nc.scalar.activation(nt_[:], xt[:], AF.Identity, bias=nmr[:], scale=rstd[:])
                psA = p0ps.tile([128, 512], F32, tag="psA")
                psB = p0ps.tile([128, 512], F32, tag="psB")
                for c in range(NC):
                    dst = psA if c < 4 else psB
                    nc.tensor.transpose(dst[:, _ts(c % 4, 128)], nt_[:, _ts(c, 128)], idt[:])
                nT = p0.tile([128, NC * 128], F32, tag="nT")
                nTv = nT[:].rearrange("p (c t) -> p c t", c=NC)
                nc.scalar.copy(nTv[:, 0:4, :], psA[:].rearrange("p (c t) -> p c t", c=4))
                nc.scalar.copy(nTv[:, 4:8, :], psB[:].rearrange("p (c t) -> p c t", c=4))
                nc.vector.tensor_tensor(out=xmr_v[:, :, i * 128 + 1:(i + 1) * 128],
                                        in0=nTv[:, :, 1:128], in1=nTv[:, :, 0:127],
                                        op=AluOpType.add)
                nc.vector.tensor_tensor(out=xmr_v[:, :, i * 128:i * 128 + 1],
                                        in0=nTv[:, :, 0:1], in1=pcv,
                                        op=AluOpType.add)
                nc.vector.tensor_copy(pcv, nTv[:, :, 127:128])

        # ---------------- P1: projections ----------------
        def load_w(stg_pool, dst_pool, name, tag):
            wrt = dst_pool.tile([128, 8 * 1024], F32R, tag=tag, name=f"w_{name}")
            for q in range(4):
                s = stg_pool.tile([128, 2048], F32, tag="wstg", name=f"ws_{name}_{q}")
                nc.sync.dma_start(
                    s[:].rearrange("p (k n) -> p k n", k=2),
                    w_dram[name].rearrange("(k p) n -> p k n", p=128)[:, 2 * q:2 * q + 2, :])
                nc.vector.tensor_copy(wrt[:, _ts(q, 2048)], s[:])
            return wrt[:].rearrange("p (k n) -> p k n", k=8)

        def proj_mms(ps_pool, wv, m, tagset):
            pts = [ps_pool.tile([128, 512], F32, tag=f"{tagset}{n}", name=f"{tagset}_{m}_{n}")
                   for n in range(4)]
            for k in range(8):
                for n in range(4):
                    nc.tensor.matmul(pts[n][:], wv[:, k, _ts(m, 128)],
                                     xmr_v[:, k, _ts(n, 512)],
                                     start=(k == 0), stop=(k == 7))
            return pts

        with ExitStack() as ctx:
            wstg = ctx.enter_context(tc.tile_pool(name="wstg", bufs=2))
            wpa = ctx.enter_context(tc.tile_pool(name="wpa", bufs=1))
            wpb = ctx.enter_context(tc.tile_pool(name="wpb", bufs=1))
            ev = ctx.enter_context(tc.tile_pool(name="ev", bufs=3))
            ps1 = ctx.enter_context(tc.tile_pool(name="ps1", bufs=1, space="PSUM"))

            wrv = load_w(wstg, wpa, "wr", "wA")
            wgv = load_w(wstg, wpb, "wg", "wB")
            for m in range(NC):
                ptr = proj_mms(ps1, wrv, m, "pA")
                ptg = proj_mms(ps1, wgv, m, "pB")
                for n in range(4):
                    sr = ev.tile([128, 512], F32, tag="e1", name=f"sr_{m}_{n}")
                    nc.scalar.activation(sr[:], ptr[n][:], AF.Sigmoid, bias=BR[:, m:m + 1])
                    sg = ev.tile([128, 512], F32, tag="e2", name=f"sg_{m}_{n}")
                    nc.scalar.activation(sg[:], ptg[n][:], AF.Sigmoid, bias=BG[:, m:m + 1])
                    srg = ev.tile([128, 512], F32, tag="e3", name=f"srg_{m}_{n}")
                    nc.vector.tensor_tensor(out=srg[:], in0=sr[:], in1=sg[:],
                                            op=AluOpType.mult)
                    nc.sync.dma_start(srg_d[_ts(m, 128), _ts(n, 512)], srg[:])

            wkv = load_w(wstg, wpa, "wk", "wA")
            wvv = load_w(wstg, wpb, "wv", "wB")
            for m in range(NC):
                ptk = proj_mms(ps1, wkv, m, "pA")
                ptv = proj_mms(ps1, wvv, m, "pB")
                for n in range(4):
                    ks = ev.tile([128, 512], F32, tag="e1", name=f"ks_{m}_{n}")
                    nc.scalar.activation(ks[:], ptk[n][:], AF.Identity, bias=BK[:, m:m + 1])
                    u1 = ev.tile([128, 512], F32, tag="e3", name=f"u1_{m}_{n}")
                    nc.vector.scalar_tensor_tensor(out=u1[:], in0=ptv[n][:],
                                                   scalar=BV[:, m:m + 1], in1=ks[:],
                                                   op0=AluOpType.add, op1=AluOpType.mult)
                    nc.sync.dma_start(u1_d[_ts(m, 128), _ts(n, 512)], u1[:])

            wav = load_w(wstg, wpa, "wa", "wA")
            wbv = load_w(wstg, wpb, "wb", "wB")
            for m in range(NC):
                pta = proj_mms(ps1, wav, m, "pA")
                ptb = proj_mms(ps1, wbv, m, "pB")
                for n in range(4):
                    as_ = ev.tile([128, 512], F32, tag="e1", name=f"as_{m}_{n}")
                    nc.scalar.activation(as_[:], pta[n][:], AF.Identity, bias=BA[:, m:m + 1])
                    ab = ev.tile([128, 512], F32, tag="e2", name=f"ab_{m}_{n}")
                    nc.vector.scalar_tensor_tensor(out=ab[:], in0=ptb[n][:],
                                                   scalar=BB[:, m:m + 1], in1=as_[:],
                                                   op0=AluOpType.add, op1=AluOpType.mult)
                    u1b = ev.tile([128, 512], F32, tag="e4", name=f"u1b_{m}_{n}")
                    nc.sync.dma_start(u1b[:], u1_d[_ts(m, 128), _ts(n, 512)])
                    ut = ev.tile([128, 512], F32, tag="e3", name=f"ut_{m}_{n}")
                    nc.vector.tensor_tensor(out=ut[:], in0=ab[:], in1=u1b[:],
                                            op=AluOpType.add)
                    nc.sync.dma_start(u_d[_ts(m, 128), _ts(n, 512)], ut[:])

        # slot B rotates: states -> mr -> kf (created after P1 pools exit)
        big2 = top.enter_context(tc.tile_pool(name="big2", bufs=1))

        # ---------------- P2: scan ----------------
        states = big2.tile([128, NC * T], F32, tag="slotB", name="states")
        states_v = states[:].rearrange("p (c t) -> p c t", c=NC)
        with ExitStack() as ctx:
            p2 = ctx.enter_context(tc.tile_pool(name="p2", bufs=2))
            for c in range(NC):
                uc = p2.tile([128, T], F32, tag="uc")
                nc.sync.dma_start(uc[:], u_d[_ts(c, 128), :])
                nc.vector.tensor_tensor_scan(states_v[:, c, :],
                                             WSCAN[:, c:c + 1].broadcast_to((128, T)),
                                             uc[:], STATE0[:, c:c + 1],
                                             AluOpType.mult, AluOpType.add)
                nc.sync.dma_start(out_fs[:, c:c + 1], states_v[:, c, T - 1:T])

        # ---------------- P3: GN + gate ----------------
        normT = big1.tile([128, NC * T], F32, tag="slotA", name="normT")
        normT_v = normT[:].rearrange("p (c t) -> p c t", c=NC)
        with ExitStack() as ctx:
            p3 = ctx.enter_context(tc.tile_pool(name="p3", bufs=2))
            p3s = ctx.enter_context(tc.tile_pool(name="p3s", bufs=2))
            p3ps = ctx.enter_context(tc.tile_pool(name="p3ps", bufs=2, space="PSUM"))
            for i in range(NT):
                psA = p3ps.tile([128, 512], F32, tag="tA")
                psB = p3ps.tile([128, 512], F32, tag="tB")
                for c in range(NC):
                    dst = psA if c < 4 else psB
                    nc.tensor.transpose(dst[:, _ts(c % 4, 128)],
                                        states_v[:, c, _ts(i, 128)], idt[:])
                nrm = p3.tile([128, D], F32, tag="nrm")
                for half, ps_ in enumerate((psA, psB)):
                    psg = ps_[:].rearrange("p (g e) -> p g e", g=16)
                    sums = p3s.tile([128, 16], F32, tag="gsum")
                    nc.vector.tensor_reduce(sums[:], psg, mybir.AxisListType.X,
                                            AluOpType.add)
                    sqs = p3s.tile([128, 512], F32, tag="gsq")
                    nc.scalar.activation(sqs[:], ps_[:], AF.Square)
                    sqsum = p3s.tile([128, 16], F32, tag="gsqs")
                    nc.vector.tensor_reduce(sqsum[:],
                                            sqs[:].rearrange("p (g e) -> p g e", g=16),
                                            mybir.AxisListType.X, AluOpType.add)
                    mean = p3s.tile([128, 16], F32, tag="gmean")
                    nc.vector.tensor_scalar(out=mean[:], in0=sums[:], scalar1=1.0 / 32,
                                            scalar2=None, op0=AluOpType.mult)
                    msq = p3s.tile([128, 16], F32, tag="gmsq")
                    nc.vector.tensor_tensor(out=msq[:], in0=mean[:], in1=mean[:],
                                            op=AluOpType.mult)
                    v = p3s.tile([128, 16], F32, tag="gv")
                    nc.vector.scalar_tensor_tensor(out=v[:], in0=sqsum[:],
                                                   scalar=1.0 / 32, in1=msq[:],
                                                   op0=AluOpType.mult,
                                                   op1=AluOpType.subtract)
                    nc.vector.tensor_scalar(out=v[:], in0=v[:], scalar1=EPS_GN,
                                            scalar2=None, op0=AluOpType.add)
                    nc.vector.reciprocal(v[:], v[:])
                    rstd = p3s.tile([128, 16], F32, tag="grstd")
                    nc.scalar.activation(rstd[:], v[:], AF.Sqrt)
                    rstd3 = rstd[:].rearrange("p (g o) -> p g o", o=1)
                    nmr = p3s.tile([128, 16], F32, tag="gnmr")
                    nmr3 = nmr[:].rearrange("p (g o) -> p g o", o=1)
                    nc.vector.scalar_tensor_tensor(out=nmr3,
                                                   in0=mean[:].rearrange(
                                                       "p (g o) -> p g o", o=1),
                                                   scalar=-1.0, in1=rstd3,
                                                   op0=AluOpType.mult,
                                                   op1=AluOpType.mult)
                    hv = nrm[:, _ts(half, 512)].rearrange("p (g e) -> p g e", g=16)
                    nc.vector.tensor_tensor(out=hv,
                                            in0=ps_[:].rearrange("p (g e) -> p g e", g=16),
                                            in1=rstd3.broadcast_to((128, 16, 32)),
                                            op=AluOpType.mult)
                    nc.vector.tensor_tensor(out=hv, in0=hv,
                                            in1=nmr3.broadcast_to((128, 16, 32)),
                                            op=AluOpType.add)
                psC = p3ps.tile([128, 512], F32, tag="tC")
                psD = p3ps.tile([128, 512], F32, tag="tD")
                for c in range(NC):
                    dst = psC if c < 4 else psD
                    nc.tensor.transpose(dst[:, _ts(c % 4, 128)], nrm[:, _ts(c, 128)], idt[:])
                nc.scalar.copy(normT_v[:, 0:4, _ts(i, 128)],
                               psC[:].rearrange("p (c t) -> p c t", c=4))
                nc.scalar.copy(normT_v[:, 4:8, _ts(i, 128)],
                               psD[:].rearrange("p (c t) -> p c t", c=4))

        mr = big2.tile([128, NC * T], F32R, tag="slotB", name="mr")
        mr_v = mr[:].rearrange("p (c t) -> p c t", c=NC)
        with ExitStack() as ctx:
            pg_ = ctx.enter_context(tc.tile_pool(name="pg", bufs=2))
            for c in range(NC):
                srgc = pg_.tile([128, T], F32, tag="srgc")
                nc.sync.dma_start(srgc[:], srg_d[_ts(c, 128), :])
                if use_gnb:
                    tmp = pg_.tile([128, T], F32, tag="gtmp")
                    nc.vector.tensor_scalar(out=tmp[:], in0=normT_v[:, c, :],
                                            scalar1=GNW[:, c:c + 1], scalar2=GNB[:, c:c + 1],
                                            op0=AluOpType.mult, op1=AluOpType.add)
                    nc.vector.tensor_tensor(out=mr_v[:, c, :], in0=tmp[:], in1=srgc[:],
                                            op=AluOpType.mult)
                else:
                    nc.vector.scalar_tensor_tensor(out=mr_v[:, c, :], in0=normT_v[:, c, :],
                                                   scalar=GNW[:, c:c + 1], in1=srgc[:],
                                                   op0=AluOpType.mult, op1=AluOpType.mult)

        # ---------------- P4: mixed/res/LN2/xx2 ----------------
        xx2 = big1.tile([128, NC * T], F32R, tag="slotA", name="xx2")
        xx2_v = xx2[:].rearrange("p (c t) -> p c t", c=NC)
        with ExitStack() as ctx:
            wstg4 = ctx.enter_context(tc.tile_pool(name="wstg4", bufs=2))
            wp4 = ctx.enter_context(tc.tile_pool(name="wp4", bufs=1))
            p4 = ctx.enter_context(tc.tile_pool(name="p4", bufs=2))
            p4s = ctx.enter_context(tc.tile_pool(name="p4s", bufs=2))
            p4ps = ctx.enter_context(tc.tile_pool(name="p4ps", bufs=2, space="PSUM"))
            woutv = load_w(wstg4, wp4, "wout", "wO")
            for i in range(NT):
                pm = p4ps.tile([128, 1024], F32, tag="pm")
                for k in range(8):
                    for n in range(2):
                        nc.tensor.matmul(pm[:, _ts(n, 512)], mr_v[:, k, _ts(i, 128)],
                                         woutv[:, k, _ts(n, 512)],
                                         start=(k == 0), stop=(k == 7))
                xt = p4.tile([128, D], F32, tag="xt4")
                nc.sync.dma_start(xt[:], x_in[_ts(i, 128), :])
                res = p4.tile([128, D], F32, tag="res")
                nc.vector.tensor_tensor(out=res[:], in0=pm[:], in1=xt[:], op=AluOpType.add)
                nc.sync.dma_start(res_d[_ts(i, 128), :], res[:])
                n2 = p4.tile([128, D], F32, tag="n2")
                rstd, nmr = ln_rstd(p4s, res[:], n2[:], "l2")
                nc.scalar.activation(n2[:], res[:], AF.Identity, bias=nmr[:], scale=rstd[:])
                psC = p4ps.tile([128, 512], F32, tag="tC4")
                psD = p4ps.tile([128, 512], F32, tag="tD4")
                for c in range(NC):
                    dst = psC if c < 4 else psD
                    nc.tensor.transpose(dst[:, _ts(c % 4, 128)], n2[:, _ts(c, 128)], idt[:])
                nc.scalar.copy(xx2_v[:, 0:4, _ts(i, 128)],
                               psC[:].rearrange("p (c t) -> p c t", c=4))
                nc.scalar.copy(xx2_v[:, 4:8, _ts(i, 128)],
                               psD[:].rearrange("p (c t) -> p c t", c=4))

        # ---------------- P5: FFN ----------------
        kf = big2.tile([128, NC * T], F32R, tag="slotB", name="kf")
        kf_v = kf[:].rearrange("p (c t) -> p c t", c=NC)
        with ExitStack() as ctx:
            wstg5 = ctx.enter_context(tc.tile_pool(name="wstg5", bufs=2))
            wp5 = ctx.enter_context(tc.tile_pool(name="wp5", bufs=1))
            ev5 = ctx.enter_context(tc.tile_pool(name="ev5", bufs=3))
            ps5 = ctx.enter_context(tc.tile_pool(name="ps5", bufs=2, space="PSUM"))

            def proj5(wv, m, tagset):
                pts = [ps5.tile([128, 512], F32, tag=f"{tagset}{n}", name=f"{tagset}_{m}_{n}")
                       for n in range(4)]
                for k in range(8):
                    for n in range(4):
                        nc.tensor.matmul(pts[n][:], wv[:, k, _ts(m, 128)],
                                         xx2_v[:, k, _ts(n, 512)],
                                         start=(k == 0), stop=(k == 7))
                return pts

            wfkv = load_w(wstg5, wp5, "wfk", "wF")
            for m in range(NC):
                pts = proj5(wfkv, m, "q")
                for n in range(4):
                    rl = ev5.tile([128, 512], F32, tag="rl", name=f"rl_{m}_{n}")
                    nc.scalar.activation(rl[:], pts[n][:], AF.Relu, bias=BFK[:, m:m + 1])
                    nc.vector.tensor_tensor(out=kf_v[:, m, _ts(n, 512)], in0=rl[:],
                                            in1=rl[:], op=AluOpType.mult)
            wfrv = load_w(wstg5, wp5, "wfr", "wF")
            for m in range(NC):
                pts = proj5(wfrv, m, "q")
                for n in range(4):
                    fr = ev5.tile([128, 512], F32, tag="rl", name=f"fr_{m}_{n}")
                    nc.scalar.activation(fr[:], pts[n][:], AF.Sigmoid, bias=BFR[:, m:m + 1])
                    nc.sync.dma_start(fr_d[_ts(m, 128), _ts(n, 512)], fr[:])

        g2 = big1.tile([128, NC * T], F32R, tag="slotA", name="g2")
        g2_v = g2[:].rearrange("p (c t) -> p c t", c=NC)
        with ExitStack() as ctx:
            wstg6 = ctx.enter_context(tc.tile_pool(name="wstg6", bufs=2))
            wp6 = ctx.enter_context(tc.tile_pool(name="wp6", bufs=1))
            ev6 = ctx.enter_context(tc.tile_pool(name="ev6", bufs=2))
            ps6 = ctx.enter_context(tc.tile_pool(name="ps6", bufs=2, space="PSUM"))
            wfvv = load_w(wstg6, wp6, "wfv", "wV")
            for m in range(NC):
                pts = [ps6.tile([128, 512], F32, tag=f"h{n}", name=f"h_{m}_{n}")
                       for n in range(4)]
                for k in range(8):
                    for n in range(4):
                        nc.tensor.matmul(pts[n][:], wfvv[:, k, _ts(m, 128)],
                                         kf_v[:, k, _ts(n, 512)],
                                         start=(k == 0), stop=(k == 7))
                for n in range(4):
                    hn = ev6.tile([128, 512], F32, tag="hn", name=f"hn_{m}_{n}")
                    nc.scalar.activation(hn[:], pts[n][:], AF.Identity, bias=BFV[:, m:m + 1])
                    if use_grn:
                        nc.sync.dma_start(h_d[_ts(m, 128), _ts(n, 512)], hn[:])
                    else:
                        frc = ev6.tile([128, 512], F32, tag="frc", name=f"frc_{m}_{n}")
                        nc.sync.dma_start(frc[:], fr_d[_ts(m, 128), _ts(n, 512)])
                        nc.vector.tensor_tensor(out=g2_v[:, m, _ts(n, 512)], in0=hn[:],
                                                in1=frc[:], op=AluOpType.mult)
            if use_grn:
                import concourse.bass_isa as bass_isa
                grn = ctx.enter_context(tc.tile_pool(name="grn", bufs=1))
                grn2 = ctx.enter_context(tc.tile_pool(name="grn2", bufs=2))
                gx = grn.tile([128, NC], F32)
                scr = grn.tile([128, T], F32)
                for m in range(NC):
                    hc = grn2.tile([128, T], F32, tag="hc", name=f"hst_{m}")
                    nc.sync.dma_start(hc[:], h_d[_ts(m, 128), :])
                    nc.vector.tensor_tensor_reduce(out=scr[:], in0=hc[:], in1=hc[:],
                                                   scale=1.0, scalar=0.0,
                                                   op0=AluOpType.mult, op1=AluOpType.add,
                                                   accum_out=gx[:, m:m + 1])
                nc.scalar.activation(gx[:], gx[:], AF.Sqrt)
                gsum = grn.tile([128, NC], F32)
                nc.gpsimd.partition_all_reduce(gsum[:], gx[:], channels=128,
                                               reduce_op=bass_isa.ReduceOp.add)
                tot = grn.tile([128, 1], F32)
                nc.vector.tensor_reduce(tot[:], gsum[:], mybir.AxisListType.X,
                                        AluOpType.add)
                nc.vector.tensor_scalar(out=tot[:], in0=tot[:], scalar1=1.0 / D,
                                        scalar2=1e-6, op0=AluOpType.mult, op1=AluOpType.add)
                nc.vector.reciprocal(tot[:], tot[:])
                nx = grn.tile([128, NC], F32)
                nc.vector.tensor_scalar(out=nx[:], in0=gx[:], scalar1=tot[:],
                                        scalar2=None, op0=AluOpType.mult)
                for m in range(NC):
                    hc = grn2.tile([128, T], F32, tag="hc", name=f"hap_{m}")
                    nc.sync.dma_start(hc[:], h_d[_ts(m, 128), :])
                    frc = grn2.tile([128, T], F32, tag="frcg", name=f"frcg_{m}")
                    nc.sync.dma_start(frc[:], fr_d[_ts(m, 128), :])
                    t1 = grn2.tile([128, T], F32, tag="t1g", name=f"t1g_{m}")
                    nc.vector.tensor_scalar(out=t1[:], in0=hc[:], scalar1=nx[:, m:m + 1],
                                            scalar2=GAMMA[:, m:m + 1],
                                            op0=AluOpType.mult, op1=AluOpType.mult)
                    nc.vector.tensor_scalar(out=t1[:], in0=t1[:], scalar1=BETA[:, m:m + 1],
                                            scalar2=None, op0=AluOpType.add)
                    nc.vector.tensor_tensor(out=t1[:], in0=t1[:], in1=hc[:],
                                            op=AluOpType.add)
                    nc.vector.tensor_tensor(out=g2_v[:, m, :], in0=t1[:], in1=frc[:],
                                            op=AluOpType.mult)

        with ExitStack() as ctx:
            wstg7 = ctx.enter_context(tc.tile_pool(name="wstg7", bufs=2))
            wp7 = ctx.enter_context(tc.tile_pool(name="wp7", bufs=1))
            p7 = ctx.enter_context(tc.tile_pool(name="p7", bufs=2))
            ps7 = ctx.enter_context(tc.tile_pool(name="ps7", bufs=2, space="PSUM"))
            wffnv = load_w(wstg7, wp7, "wffn", "wN")
            for i in range(NT):
                pm = ps7.tile([128, 1024], F32, tag="pm7")
                for k in range(8):
                    for n in range(2):
                        nc.tensor.matmul(pm[:, _ts(n, 512)], g2_v[:, k, _ts(i, 128)],
                                         wffnv[:, k, _ts(n, 512)],
                                         start=(k == 0), stop=(k == 7))
                resc = p7.tile([128, D], F32, tag="resc")
                nc.sync.dma_start(resc[:], res_d[_ts(i, 128), :])
                fo = p7.tile([128, D], F32, tag="fo")
                nc.vector.tensor_tensor(out=fo[:], in0=pm[:], in1=resc[:], op=AluOpType.add)
                nc.sync.dma_start(out_x[_ts(i, 128), :], fo[:])

    nc.compile()
    return nc


def _to_cmajor(v):
    return np.ascontiguousarray(np.asarray(v, np.float32).reshape(NC, 128).T)


def kernel(x, state, ln1_w, ln1_b, ln2_w, ln2_b, gn_w, gn_b, grn_gamma, grn_beta,
           decay, Wr, Wk, Wv, Wg, Wa, Wb, Wout, Wffnout, Wfk, bfk, Wfv, bfv, Wfr, bfr):
    f = lambda a: np.asarray(a, dtype=np.float32)
    x = f(x); state = f(state)
    ln1_w, ln1_b, ln2_w, ln2_b = f(ln1_w), f(ln1_b), f(ln2_w), f(ln2_b)
    gn_w, gn_b = f(gn_w), f(gn_b)
    grn_gamma, grn_beta = f(grn_gamma).reshape(-1), f(grn_beta).reshape(-1)
    decay = f(decay)
    Wr, Wk, Wv, Wg, Wa, Wb = f(Wr), f(Wk), f(Wv), f(Wg), f(Wa), f(Wb)
    Wout, Wffnout, Wfk, Wfv, Wfr = f(Wout), f(Wffnout), f(Wfk), f(Wfv), f(Wfr)
    bfk, bfv, bfr = f(bfk), f(bfv), f(bfr)

    use_grn = bool(np.any(grn_gamma) or np.any(grn_beta))
    use_gnb = bool(np.any(gn_b))
    key = (use_grn, use_gnb)
    if key not in _CACHE:
        _CACHE[key] = _build(use_grn, use_gnb)
    nc = _CACHE[key]

    half_w = 0.5 * ln1_w
    foldp = lambda W: np.ascontiguousarray(half_w[:, None] * W)
    wr_, wk_, wv_, wa_, wb_, wg_ = map(foldp, (Wr, Wk, Wv, Wa, Wb, Wg))
    br_, bk_, bv_ = ln1_b @ Wr, ln1_b @ Wk, ln1_b @ Wv
    ba_, bb_, bg_ = ln1_b @ Wa, ln1_b @ Wb, ln1_b @ Wg
    wfk_ = np.ascontiguousarray(ln2_w[:, None] * Wfk)
    bfk_ = ln2_b @ Wfk + bfk
    wfr_ = np.ascontiguousarray(ln2_w[:, None] * Wfr)
    bfr_ = ln2_b @ Wfr + bfr

    w_scan = np.exp(-np.exp(decay)).astype(np.float32)
    with np.errstate(divide="ignore", invalid="ignore"):
        pcol = np.where(np.abs(ln1_w) > 1e-30, -ln1_b / np.where(ln1_w == 0, 1, ln1_w),
                        0.0).astype(np.float32)

    consts = np.zeros((128, 64), np.float32)
    consts[:, 0:8] = _to_cmajor(pcol)
    consts[:, 8:16] = _to_cmajor(w_scan)
    consts[:, 24:32] = _to_cmajor(gn_w)
    consts[:, 32:40] = _to_cmajor(gn_b)
    consts[:, 40:48] = _to_cmajor(bk_)
    consts[:, 48:56] = _to_cmajor(bv_)
    consts[:, 56:64] = _to_cmajor(grn_beta)
    consts2 = np.zeros((128, 64), np.float32)
    consts2[:, 0:8] = _to_cmajor(ba_)
    consts2[:, 8:16] = _to_cmajor(bb_)
    consts2[:, 16:24] = _to_cmajor(br_)
    consts2[:, 24:32] = _to_cmajor(bg_)
    consts2[:, 32:40] = _to_cmajor(bfk_)
    consts2[:, 40:48] = _to_cmajor(bfr_)
    consts2[:, 48:56] = _to_cmajor(bfv)
    consts2[:, 56:64] = _to_cmajor(grn_gamma)
    ident = np.eye(128, dtype=np.float32)

    in_maps = []
    for b in range(B):
        cm = consts.copy()
        cm[:, 16:24] = _to_cmajor(state[b])
        in_maps.append({
            "x_in": np.ascontiguousarray(x[b]),
            "consts": cm, "consts2": consts2, "ident": ident,
            "wr": wr_, "wk": wk_, "wv": wv_, "wa": wa_, "wb": wb_, "wg": wg_,
            "wout": Wout, "wfk": wfk_, "wfr": wfr_, "wfv": Wfv, "wffn": Wffnout,
        })

    global _LAST_IN_MAPS
    _LAST_IN_MAPS = in_maps
    res = bass_utils.run_bass_kernel_spmd(nc, in_maps, core_ids=list(range(B)))
    x_out = np.stack([res.results[b]["out_x"] for b in range(B)])
    fs = np.stack([res.results[b]["out_fs"].T.reshape(D) for b in range(B)])
    return x_out, fs
